# revision 2
# baseline (speedup 1.0000x reference)
"""GCC-PHAT Trainium2 kernel (v3: radix-4 factored forward DFT).

Pipeline (per core, batch-sharded B=16 -> 2 per core):
  1. Stage-1 radix-4 butterflies (DVE/Pool adds, fp16): with
     n = p + 128*(c1 + 2*n2)  (= n1 + 256*n2, n1 = p + 128*c1), compute
     V0 = sum_n2 x, V2 = alternating sum, PR = x(n2=0)-x(n2=2),
     PI = x(n2=1)-x(n2=3)   (V1 = PR - i*PI, V3 = conj V1).
  2. Stage-2 DFT matmuls (PE, fp16, twiddles absorbed into 24 precomputed
     [128x128] weight mats): chunk c holds freqs f = 4r+c+1 (r=0..127);
     outputs ps_a = Re X, ps_b = Im X per chunk.  f=512 lands naturally in
     chunk 3 row 127 (its Im row is all-zero).  Bin 0 (DC) is handled on
     the host via a sign product (PHAT reduces it to sign(S1)*sign(S2)/L).
  3. PHAT normalize per mic: w' = 1/sqrt(16*(a^2+b^2)) via ACT
     Abs_reciprocal_sqrt; ya = a*w', yb = b*w' (unit/4), ys = ya+yb,
     yd = ya-yb, all fp16.
  4. Pair products (28 mic pairs, diagonal pairing): Karatsuba planes
     k1 = ys1*a2, k2 = a1*ys2, k3 = b1*yd2 per chunk -> 12 planes fp16
     (DVE: k1,k2; Pool: k3).
  5. Truncated inverse DFT as PE matmul, G stationary: 12 K-chunks of
     [128f x 64 lags] accumulated into PSUM; G rows carry 16x scale +
     irfft weights/fftshift/slice.
  6. PSUM -> ACT copy -> SBUF -> DMA to out[b, lag, p, t] (lag-major;
     host transposes back and adds the DC term).
"""

import os
from contextlib import ExitStack

import numpy as np

import concourse.bass as bass
import concourse.bacc as bacc
import concourse.mybir as mybir
import concourse.tile as tile
from concourse.bass import ds, ts
from concourse.bass_utils import run_bass_kernel_spmd

B, M, T, L = 16, 8, 250, 1024
NCORES = 8
NB = B // NCORES          # batches per core
NPAIRS = (M * (M - 1)) // 2   # 28
NL = 64                   # output lags
F32 = mybir.dt.float32
FP16 = mybir.dt.float16
LANE_CAP = 6              # max lanes per pair group (SBUF-driven)


def _build_W():
    """24 stage-2 weight mats [k=128, m=128] + schedule.

    Returns (W [24,128,128] fp16, sched) where sched[(c, slot)] is a list of
    (widx, plane, c1); slot 0 = Re chunk (ps_a), 1 = Im chunk (ps_b).
    Planes: 0=V0, 1=V2, 2=PR, 3=PI.
    """
    mats, sched = [], {}
    for c in range(4):
        f = 4 * np.arange(128, dtype=np.float64) + c + 1
        ent_a, ent_b = [], []
        for c1 in range(2):
            n1 = 128 * c1 + np.arange(128, dtype=np.float64)
            th = 2 * np.pi * np.outer(n1, f) / L
            cos, sin = np.cos(th), np.sin(th)
            if c == 3:      # f0 = 0: V0
                ea, eb = [(0, c1, cos)], [(0, c1, -sin)]
            elif c == 1:    # f0 = 2: V2
                ea, eb = [(1, c1, cos)], [(1, c1, -sin)]
            elif c == 0:    # f0 = 1: V1 = PR - i*PI
                ea = [(2, c1, cos), (3, c1, -sin)]
                eb = [(2, c1, -sin), (3, c1, -cos)]
            else:           # c == 2, f0 = 3: V3 = PR + i*PI
                ea = [(2, c1, cos), (3, c1, sin)]
                eb = [(2, c1, -sin), (3, c1, cos)]
            ent_a.extend(ea)
            ent_b.extend(eb)
        for slot, ents in ((0, ent_a), (1, ent_b)):
            lst = []
            for plane, c1, mat in ents:
                lst.append((len(mats), plane, c1))
                mats.append(mat)
            sched[(c, slot)] = lst
    W = np.stack(mats).astype(np.float16)
    return W, sched


_W_NP, _W_SCHED = _build_W()


def _build_G() -> np.ndarray:
    """12 inverse planes [128, 64]: idx c = k1, 4+c = k2, 8+c = k3."""
    G = np.zeros((12, 128, NL), dtype=np.float64)
    nj = (np.arange(NL) - 32).astype(np.float64)
    for c in range(4):
        for r in range(128):
            f = 4 * r + c + 1
            w = 1.0 if f == 512 else 2.0
            cosv = 16.0 * w * np.cos(2 * np.pi * f * nj / L) / L
            sinv = 16.0 * w * np.sin(2 * np.pi * f * nj / L) / L
            G[0 + c, r] = cosv - sinv     # k1 = ys1*a2
            G[4 + c, r] = sinv            # k2 = a1*ys2
            G[8 + c, r] = -cosv           # k3 = b1*yd2
    return G.astype(np.float16)


def build_bass() -> bass.Bass:
    nc = bacc.Bacc("TRN2", target_bir_lowering=False, debug=False)
    xT = nc.dram_tensor("xT", [NB, M, L, T], FP16, kind="ExternalInput")
    out = nc.dram_tensor("out", [NB, NL, NPAIRS, T], F32, kind="ExternalOutput")
    Wh = nc.inline_tensor(np.ascontiguousarray(_W_NP), name="Wmat")
    Gh = nc.inline_tensor(np.ascontiguousarray(_build_G()), name="Gmat")

    with tile.TileContext(nc) as tc, ExitStack() as ctx:
        consts = ctx.enter_context(tc.tile_pool(name="consts", bufs=1))
        xt_pool = ctx.enter_context(tc.tile_pool(name="xt", bufs=2))
        v_pool = ctx.enter_context(tc.tile_pool(name="v", bufs=2))
        tb_pool = ctx.enter_context(tc.tile_pool(name="tb", bufs=2))
        y_pool = ctx.enter_context(tc.tile_pool(name="y", bufs=1))
        tmp_pool = ctx.enter_context(tc.tile_pool(name="tmp", bufs=2))
        r_pool = ctx.enter_context(tc.tile_pool(name="r", bufs=2))
        fwd_psum = ctx.enter_context(tc.tile_pool(name="fps", bufs=2, space="PSUM"))
        inv_psum = ctx.enter_context(tc.tile_pool(name="ips", bufs=2, space="PSUM"))

        w_sb = consts.tile([128, 24, 128], FP16)
        nc.sync.dma_start(w_sb[:], Wh[:].rearrange("i p j -> p i j"))
        g_sb = consts.tile([128, 12, NL], FP16)
        nc.sync.dma_start(g_sb[:], Gh[:].rearrange("i p j -> p i j"))

        for b in range(NB):
            # Y tiles: [128, mg(4), m(2), t] fp16 per chunk
            ya = [y_pool.tile([128, 4, 2, T], FP16, tag=f"ya{c}", name=f"ya{c}") for c in range(4)]
            yb = [y_pool.tile([128, 4, 2, T], FP16, tag=f"yb{c}", name=f"yb{c}") for c in range(4)]
            ys = [y_pool.tile([128, 4, 2, T], FP16, tag=f"ys{c}", name=f"ys{c}") for c in range(4)]
            yd = [y_pool.tile([128, 4, 2, T], FP16, tag=f"yd{c}", name=f"yd{c}") for c in range(4)]

            for mg in range(4):
                xt_sb = xt_pool.tile([128, 8, 2, T], FP16, tag="xt")
                for mi in range(2):
                    nc.sync.dma_start(
                        xt_sb[:, :, mi],
                        xT[b, 2 * mg + mi].rearrange("(k p) t -> p k t", p=128),
                    )
                # ---- stage-1 radix-4 butterflies ----
                v_sb = v_pool.tile([128, 4, 2, 2, T], FP16, tag="v")  # plane, c1, mi, t
                tb_sb = tb_pool.tile([128, 2, 2, 2, T], FP16, tag="tb")  # c1, b01, mi, t
                for c1 in range(2):
                    nc.vector.tensor_add(tb_sb[:, c1, 0], xt_sb[:, c1], xt_sb[:, c1 + 4])
                    nc.vector.tensor_add(tb_sb[:, c1, 1], xt_sb[:, c1 + 2], xt_sb[:, c1 + 6])
                    nc.gpsimd.tensor_sub(v_sb[:, 2, c1], xt_sb[:, c1], xt_sb[:, c1 + 4])
                    nc.gpsimd.tensor_sub(v_sb[:, 3, c1], xt_sb[:, c1 + 2], xt_sb[:, c1 + 6])
                    nc.vector.tensor_add(v_sb[:, 0, c1], tb_sb[:, c1, 0], tb_sb[:, c1, 1])
                    nc.vector.tensor_sub(v_sb[:, 1, c1], tb_sb[:, c1, 0], tb_sb[:, c1, 1])

                # ---- stage-2 DFT + normalize ----
                for c in range(4):
                    ps_a = fwd_psum.tile([128, 2, T], F32, tag="psa")
                    ps_b = fwd_psum.tile([128, 2, T], F32, tag="psb")
                    for ps, slot in ((ps_a, 0), (ps_b, 1)):
                        lst = _W_SCHED[(c, slot)]
                        for i, (widx, plane, c1) in enumerate(lst):
                            nc.tensor.matmul(
                                ps[:],
                                w_sb[:, widx],
                                v_sb[:, plane, c1],
                                start=(i == 0), stop=(i == len(lst) - 1),
                            )
                    sq_a = tmp_pool.tile([128, 2, T], F32, tag="sqa")
                    sq_b = tmp_pool.tile([128, 2, T], F32, tag="sqb")
                    w = tmp_pool.tile([128, 2, T], F32, tag="w")
                    nc.scalar.square(sq_a[:], ps_a[:])
                    nc.scalar.square(sq_b[:], ps_b[:])
                    nc.gpsimd.tensor_add(sq_a[:], sq_a[:], sq_b[:])
                    # w' = 1/sqrt(16*r) = (1/|X|)/4
                    nc.scalar.activation(
                        w[:], sq_a[:],
                        mybir.ActivationFunctionType.Abs_reciprocal_sqrt,
                        scale=16.0,
                    )
                    nc.vector.tensor_mul(ya[c][:, mg], ps_a[:], w[:])
                    nc.vector.tensor_mul(yb[c][:, mg], ps_b[:], w[:])
                    nc.vector.tensor_add(ys[c][:, mg], ya[c][:, mg], yb[c][:, mg])
                    nc.gpsimd.tensor_sub(yd[c][:, mg], ya[c][:, mg], yb[c][:, mg])

            # ---- pairs + inverse (diagonal pairing) ----
            yaf = [ya[c][:].rearrange("p a b t -> p (a b t)") for c in range(4)]
            ybf = [yb[c][:].rearrange("p a b t -> p (a b t)") for c in range(4)]
            ysf = [ys[c][:].rearrange("p a b t -> p (a b t)") for c in range(4)]
            ydf = [yd[c][:].rearrange("p a b t -> p (a b t)") for c in range(4)]
            for d in range(1, M):
                lanes = M - d
                kb = sum(M - dd for dd in range(1, d))
                for l0 in range(0, lanes, LANE_CAP):
                    lc = min(LANE_CAP, lanes - l0)
                    rows = lc * T
                    s1 = slice(l0 * T, l0 * T + rows)            # m1 side
                    s2 = slice((l0 + d) * T, (l0 + d) * T + rows)  # m2 side
                    r_sb = r_pool.tile([128, 12, LANE_CAP * T], FP16, tag="ru")
                    for c in range(4):
                        nc.vector.tensor_mul(r_sb[:, 0 + c, :rows], ysf[c][:, s1], yaf[c][:, s2])
                        nc.vector.tensor_mul(r_sb[:, 4 + c, :rows], yaf[c][:, s1], ysf[c][:, s2])
                        nc.gpsimd.tensor_mul(r_sb[:, 8 + c, :rows], ybf[c][:, s1], ydf[c][:, s2])
                    for n0 in range(0, rows, 500):
                        nn = min(500, rows - n0)
                        ps_o = inv_psum.tile([64, 500], F32, tag="ops")
                        for idx in range(12):
                            nc.tensor.matmul(
                                ps_o[:, :nn],
                                g_sb[:, idx],
                                r_sb[:, idx, ds(n0, nn)],
                                start=(idx == 0), stop=(idx == 11),
                            )
                        o_sb = tmp_pool.tile([64, 2, T], F32, tag="osb")
                        nlanes = nn // T
                        nc.scalar.copy(
                            o_sb[:, :nlanes],
                            ps_o[:, :nn].rearrange("p (l t) -> p l t", t=T),
                        )
                        nc.sync.dma_start(
                            out[b, :, ds(kb + l0 + n0 // T, nlanes)],
                            o_sb[:, :nlanes],
                        )
    nc.compile()
    return nc


_NC_CACHE = None


def kernel(x: np.ndarray) -> np.ndarray:
    global _NC_CACHE
    x = np.asarray(x, dtype=np.float32)
    assert x.shape == (B, M, T, L)
    xT = np.ascontiguousarray(x.transpose(0, 1, 3, 2)).astype(np.float16)
    s0 = np.sign(x.sum(axis=-1))  # [B, M, T] DC sign for host PHAT term
    if _NC_CACHE is None:
        _NC_CACHE = build_bass()
    nc = _NC_CACHE
    in_maps = [{"xT": xT[c * NB:(c + 1) * NB]} for c in range(NCORES)]
    trace = bool(int(os.environ.get("GCC_TRACE", "0")))
    res = run_bass_kernel_spmd(nc, in_maps, core_ids=list(range(NCORES)),
                               trace=trace)
    if trace and res.exec_time_ns is not None:
        print(f"HW exec time: {res.exec_time_ns} ns")
        if res.instructions_and_trace is not None:
            print("trace:", res.instructions_and_trace[1])
    out = np.concatenate([r["out"] for r in res.results], axis=0)  # [B,NL,28diag,T]
    plist = [m * (2 * M - m - 1) // 2 + (m + d - m - 1)
             for d in range(1, M) for m in range(M - d)]
    final = np.empty((B, NPAIRS, T, NL), dtype=np.float32)
    final[:, plist] = out.transpose(0, 2, 3, 1)
    # host DC (bin 0) PHAT term: sign(S1)*sign(S2)/L, constant over lags
    i1, i2 = np.triu_indices(M, k=1)
    final += (s0[:, i1] * s0[:, i2])[..., None].astype(np.float32) / L
    return final


# revision 3
# speedup vs baseline: 1.3154x; 1.3154x over previous
"""GCC-PHAT Trainium2 kernel (v4: hybrid direct/radix-4 forward).

Pipeline (per core, batch-sharded B=16 -> 2 per core):
  Freq chunks c=0..3 hold f = 4r+c+1 (r=0..127).  Forward:
  - c=1 (f0=2) and c=3 (f0=0): DIRECT DFT matmuls from raw x
    (8 K-passes each for Re and Im) -- no vector-engine prework, so the
    PE can start as soon as the DMA lands and stays busy (HAM-warm).
  - c=0 (f0=1) and c=2 (f0=3): radix-4 factored: Pool computes
    PR = x(n2=0)-x(n2=2), PI = x(n2=1)-x(n2=3) over n = n1 + 256*n2
    (V1 = PR - i*PI), then 4 K-passes each (twiddles absorbed into
    precomputed [128x128] mats).  48 passes/iter vs 64 direct.
  f=512 lands naturally in chunk 3 row 127 (its Im row is zero).
  Bin 0 (DC) is handled on the host: PHAT reduces it to
  sign(S1)*sign(S2)/L, constant over lags.
  PHAT normalize: w' = 1/sqrt(16(a^2+b^2)) via ACT; ya/yb/ys/yd fp16.
  Pairs: uniform Karatsuba k1 = ys1*a2, k2 = a1*ys2, k3 = b1*yd2
  -> 12 planes (DVE: k1,k2; Pool: k3), diagonal pairing, lane cap 6.
  Inverse: 12 K-chunk PE matmuls with G stationary [128f x 64 lags],
  PSUM accumulate; ACT copy -> DMA out[b, lag, p, t]; host transposes
  and adds the DC term.
"""

import os
from contextlib import ExitStack

import numpy as np

import concourse.bass as bass
import concourse.bacc as bacc
import concourse.mybir as mybir
import concourse.tile as tile
from concourse.bass import ds, ts
from concourse.bass_utils import run_bass_kernel_spmd

B, M, T, L = 16, 8, 250, 1024
NCORES = 8
NB = B // NCORES          # batches per core
NPAIRS = (M * (M - 1)) // 2   # 28
NL = 64                   # output lags
F32 = mybir.dt.float32
FP16 = mybir.dt.float16
LANE_CAP = 6              # max lanes per pair group


def _build_W():
    """Stage-2/direct weight mats [k=128, m=128] + schedule.

    sched[(c, slot)] = list of (widx, src, idx) where src is 'v' (radix
    planes: idx = (plane, c1) flattened) or 'x' (raw x: idx = k chunk);
    slot 0 = Re (ps_a), 1 = Im (ps_b).
    """
    mats, sched = [], {}

    def add(mat):
        mats.append(mat)
        return len(mats) - 1

    for c in range(4):
        f = 4 * np.arange(128, dtype=np.float64) + c + 1
        ent_a, ent_b = [], []
        if c in (1, 3):
            # direct from x: n = 128*k + p
            for k in range(8):
                n = 128 * k + np.arange(128, dtype=np.float64)
                th = 2 * np.pi * np.outer(n, f) / L
                ent_a.append(('x', k, np.cos(th)))
                ent_b.append(('x', k, -np.sin(th)))
        else:
            # radix-4: V1 = PR - i*PI; c=0 uses V1 (f0=1), c=2 uses conj V1
            sgn = 1.0 if c == 0 else -1.0
            for c1 in range(2):
                n1 = 128 * c1 + np.arange(128, dtype=np.float64)
                th = 2 * np.pi * np.outer(n1, f) / L
                cos, sin = np.cos(th), np.sin(th)
                # planes: 0 = PR, 1 = PI (within v tile); idx = plane*2 + c1
                ent_a.append(('v', 0 * 2 + c1, cos))
                ent_a.append(('v', 1 * 2 + c1, -sgn * sin))
                ent_b.append(('v', 0 * 2 + c1, -sin))
                ent_b.append(('v', 1 * 2 + c1, -sgn * cos))
        for slot, ents in ((0, ent_a), (1, ent_b)):
            sched[(c, slot)] = [(add(mat), src, idx) for src, idx, mat in ents]
    W = np.stack(mats).astype(np.float16)
    return W, sched


_W_NP, _W_SCHED = _build_W()


def _build_G() -> np.ndarray:
    """12 inverse planes [128, 64]: idx c = k1, 4+c = k2, 8+c = k3."""
    G = np.zeros((12, 128, NL), dtype=np.float64)
    nj = (np.arange(NL) - 32).astype(np.float64)
    for c in range(4):
        for r in range(128):
            f = 4 * r + c + 1
            w = 1.0 if f == 512 else 2.0
            cosv = 16.0 * w * np.cos(2 * np.pi * f * nj / L) / L
            sinv = 16.0 * w * np.sin(2 * np.pi * f * nj / L) / L
            G[0 + c, r] = cosv - sinv     # k1 = ys1*a2
            G[4 + c, r] = sinv            # k2 = a1*ys2
            G[8 + c, r] = -cosv           # k3 = b1*yd2
    return G.astype(np.float16)


def build_bass() -> bass.Bass:
    nc = bacc.Bacc("TRN2", target_bir_lowering=False, debug=False)
    xT = nc.dram_tensor("xT", [NB, M, L, T], FP16, kind="ExternalInput")
    out = nc.dram_tensor("out", [NB, NL, NPAIRS, T], F32, kind="ExternalOutput")
    Wh = nc.inline_tensor(np.ascontiguousarray(_W_NP), name="Wmat")
    Gh = nc.inline_tensor(np.ascontiguousarray(_build_G()), name="Gmat")

    with tile.TileContext(nc) as tc, ExitStack() as ctx:
        consts = ctx.enter_context(tc.tile_pool(name="consts", bufs=1))
        xt_pool = ctx.enter_context(tc.tile_pool(name="xt", bufs=3))
        v_pool = ctx.enter_context(tc.tile_pool(name="v", bufs=2))
        y_pool = ctx.enter_context(tc.tile_pool(name="y", bufs=1))
        tmp_pool = ctx.enter_context(tc.tile_pool(name="tmp", bufs=2))
        r_pool = ctx.enter_context(tc.tile_pool(name="r", bufs=2))
        fwd_psum = ctx.enter_context(tc.tile_pool(name="fps", bufs=3, space="PSUM"))
        inv_psum = ctx.enter_context(tc.tile_pool(name="ips", bufs=2, space="PSUM"))

        w_sb = consts.tile([128, len(_W_NP), 128], FP16)
        nc.sync.dma_start(w_sb[:], Wh[:].rearrange("i p j -> p i j"))
        g_sb = consts.tile([128, 12, NL], FP16)
        nc.sync.dma_start(g_sb[:], Gh[:].rearrange("i p j -> p i j"))

        for b in range(NB):
            ya = [y_pool.tile([128, 4, 2, T], FP16, tag=f"ya{c}", name=f"ya{c}") for c in range(4)]
            yb = [y_pool.tile([128, 4, 2, T], FP16, tag=f"yb{c}", name=f"yb{c}") for c in range(4)]
            ys = [y_pool.tile([128, 4, 2, T], FP16, tag=f"ys{c}", name=f"ys{c}") for c in range(4)]
            yd = [y_pool.tile([128, 4, 2, T], FP16, tag=f"yd{c}", name=f"yd{c}") for c in range(4)]

            for mg in range(4):
                xt_sb = xt_pool.tile([128, 8, 2, T], FP16, tag="xt")
                for mi in range(2):
                    nc.sync.dma_start(
                        xt_sb[:, :, mi],
                        xT[b, 2 * mg + mi].rearrange("(k p) t -> p k t", p=128),
                    )
                # radix-4 half-butterflies on Pool: n2-chunk k = c1 + 2*n2
                v_sb = v_pool.tile([128, 2, 2, 2, T], FP16, tag="v")  # plane, c1, mi, t
                for c1 in range(2):
                    nc.gpsimd.tensor_sub(v_sb[:, 0, c1], xt_sb[:, c1], xt_sb[:, c1 + 4])
                    nc.gpsimd.tensor_sub(v_sb[:, 1, c1], xt_sb[:, c1 + 2], xt_sb[:, c1 + 6])
                vf = v_sb[:].rearrange("p a b c t -> p (a b) c t")

                # direct chunks first (only need the DMA), radix chunks after
                for c in (1, 3, 0, 2):
                    ps_a = fwd_psum.tile([128, 2, T], F32, tag="psa")
                    ps_b = fwd_psum.tile([128, 2, T], F32, tag="psb")
                    for ps, slot in ((ps_a, 0), (ps_b, 1)):
                        lst = _W_SCHED[(c, slot)]
                        for i, (widx, src, idx) in enumerate(lst):
                            rhs = xt_sb[:, idx] if src == 'x' else vf[:, idx]
                            nc.tensor.matmul(
                                ps[:], w_sb[:, widx], rhs,
                                start=(i == 0), stop=(i == len(lst) - 1),
                            )
                    sq_a = tmp_pool.tile([128, 2, T], F32, tag="sqa")
                    sq_b = tmp_pool.tile([128, 2, T], F32, tag="sqb")
                    w = tmp_pool.tile([128, 2, T], F32, tag="w")
                    nc.scalar.square(sq_a[:], ps_a[:])
                    nc.scalar.square(sq_b[:], ps_b[:])
                    nc.gpsimd.tensor_add(sq_a[:], sq_a[:], sq_b[:])
                    nc.scalar.activation(
                        w[:], sq_a[:],
                        mybir.ActivationFunctionType.Abs_reciprocal_sqrt,
                        scale=16.0,
                    )
                    nc.vector.tensor_mul(ya[c][:, mg], ps_a[:], w[:])
                    nc.vector.tensor_mul(yb[c][:, mg], ps_b[:], w[:])
                    nc.vector.tensor_add(ys[c][:, mg], ya[c][:, mg], yb[c][:, mg])
                    nc.vector.tensor_sub(yd[c][:, mg], ya[c][:, mg], yb[c][:, mg])

            # ---- pairs + inverse (diagonal pairing) ----
            yaf = [ya[c][:].rearrange("p a b t -> p (a b t)") for c in range(4)]
            ybf = [yb[c][:].rearrange("p a b t -> p (a b t)") for c in range(4)]
            ysf = [ys[c][:].rearrange("p a b t -> p (a b t)") for c in range(4)]
            ydf = [yd[c][:].rearrange("p a b t -> p (a b t)") for c in range(4)]
            for d in range(1, M):
                lanes = M - d
                kb = sum(M - dd for dd in range(1, d))
                for l0 in range(0, lanes, LANE_CAP):
                    lc = min(LANE_CAP, lanes - l0)
                    rows = lc * T
                    s1 = slice(l0 * T, l0 * T + rows)            # m1 side
                    s2 = slice((l0 + d) * T, (l0 + d) * T + rows)  # m2 side
                    r_sb = r_pool.tile([128, 12, LANE_CAP * T], FP16, tag="ru")
                    for c in range(4):
                        nc.vector.tensor_mul(r_sb[:, 0 + c, :rows], ysf[c][:, s1], yaf[c][:, s2])
                        nc.vector.tensor_mul(r_sb[:, 4 + c, :rows], yaf[c][:, s1], ysf[c][:, s2])
                        nc.gpsimd.tensor_mul(r_sb[:, 8 + c, :rows], ybf[c][:, s1], ydf[c][:, s2])
                    for n0 in range(0, rows, 500):
                        nn = min(500, rows - n0)
                        ps_o = inv_psum.tile([64, 500], F32, tag="ops")
                        for idx in range(12):
                            nc.tensor.matmul(
                                ps_o[:, :nn],
                                g_sb[:, idx],
                                r_sb[:, idx, ds(n0, nn)],
                                start=(idx == 0), stop=(idx == 11),
                            )
                        o_sb = tmp_pool.tile([64, 2, T], F32, tag="osb")
                        nlanes = nn // T
                        nc.scalar.copy(
                            o_sb[:, :nlanes],
                            ps_o[:, :nn].rearrange("p (l t) -> p l t", t=T),
                        )
                        nc.sync.dma_start(
                            out[b, :, ds(kb + l0 + n0 // T, nlanes)],
                            o_sb[:, :nlanes],
                        )
    nc.compile()
    return nc


_NC_CACHE = None


def kernel(x: np.ndarray) -> np.ndarray:
    global _NC_CACHE
    x = np.asarray(x, dtype=np.float32)
    assert x.shape == (B, M, T, L)
    xT = np.ascontiguousarray(x.transpose(0, 1, 3, 2)).astype(np.float16)
    s0 = np.sign(x.sum(axis=-1))  # [B, M, T] DC sign for host PHAT term
    if _NC_CACHE is None:
        _NC_CACHE = build_bass()
    nc = _NC_CACHE
    in_maps = [{"xT": xT[c * NB:(c + 1) * NB]} for c in range(NCORES)]
    trace = bool(int(os.environ.get("GCC_TRACE", "0")))
    res = run_bass_kernel_spmd(nc, in_maps, core_ids=list(range(NCORES)),
                               trace=trace)
    if trace and res.exec_time_ns is not None:
        print(f"HW exec time: {res.exec_time_ns} ns")
        if res.instructions_and_trace is not None:
            print("trace:", res.instructions_and_trace[1])
    out = np.concatenate([r["out"] for r in res.results], axis=0)  # [B,NL,28diag,T]
    plist = [m * (2 * M - m - 1) // 2 + (m + d - m - 1)
             for d in range(1, M) for m in range(M - d)]
    final = np.empty((B, NPAIRS, T, NL), dtype=np.float32)
    final[:, plist] = out.transpose(0, 2, 3, 1)
    # host DC (bin 0) PHAT term: sign(S1)*sign(S2)/L, constant over lags
    i1, i2 = np.triu_indices(M, k=1)
    final += (s0[:, i1] * s0[:, i2])[..., None].astype(np.float32) / L
    return final


# revision 7
# speedup vs baseline: 1.3156x; 1.0002x over previous
"""GCC-PHAT Trainium2 kernel (v4: hybrid direct/radix-4 forward).

Pipeline (per core, batch-sharded B=16 -> 2 per core):
  Freq chunks c=0..3 hold f = 4r+c+1 (r=0..127).  Forward:
  - c=1 (f0=2) and c=3 (f0=0): DIRECT DFT matmuls from raw x
    (8 K-passes each for Re and Im) -- no vector-engine prework, so the
    PE can start as soon as the DMA lands and stays busy (HAM-warm).
  - c=0 (f0=1) and c=2 (f0=3): radix-4 factored: Pool computes
    PR = x(n2=0)-x(n2=2), PI = x(n2=1)-x(n2=3) over n = n1 + 256*n2
    (V1 = PR - i*PI), then 4 K-passes each (twiddles absorbed into
    precomputed [128x128] mats).  48 passes/iter vs 64 direct.
  f=512 lands naturally in chunk 3 row 127 (its Im row is zero).
  Bin 0 (DC) is handled on the host: PHAT reduces it to
  sign(S1)*sign(S2)/L, constant over lags.
  PHAT normalize: w' = 1/sqrt(16(a^2+b^2)) via ACT; ya/yb/ys/yd fp16.
  Pairs: uniform Karatsuba k1 = ys1*a2, k2 = a1*ys2, k3 = b1*yd2
  -> 12 planes (DVE: k1,k2; Pool: k3), diagonal pairing, lane cap 6.
  Inverse: 12 K-chunk PE matmuls with G stationary [128f x 64 lags],
  PSUM accumulate; ACT copy -> DMA out[b, lag, p, t]; host transposes
  and adds the DC term.
"""

import os
from contextlib import ExitStack

import numpy as np

import concourse.bass as bass
import concourse.bacc as bacc
import concourse.mybir as mybir
import concourse.tile as tile
from concourse.bass import ds, ts
from concourse.bass_utils import run_bass_kernel_spmd

B, M, T, L = 16, 8, 250, 1024
NCORES = 8
NB = B // NCORES          # batches per core
NPAIRS = (M * (M - 1)) // 2   # 28
NL = 64                   # output lags
F32 = mybir.dt.float32
FP16 = mybir.dt.float16
LANE_CAP = 2              # max lanes per pair group (small => y double-buffer fits)


def _build_W():
    """Stage-2/direct weight mats [k=128, m=128] + schedule.

    sched[(c, slot)] = list of (widx, src, idx) where src is 'v' (radix
    planes: idx = (plane, c1) flattened) or 'x' (raw x: idx = k chunk);
    slot 0 = Re (ps_a), 1 = Im (ps_b).
    """
    mats, sched = [], {}

    def add(mat):
        mats.append(mat)
        return len(mats) - 1

    for c in range(4):
        f = 4 * np.arange(128, dtype=np.float64) + c + 1
        ent_a, ent_b = [], []
        if c in (1, 3):
            # direct from x: n = 128*k + p
            for k in range(8):
                n = 128 * k + np.arange(128, dtype=np.float64)
                th = 2 * np.pi * np.outer(n, f) / L
                ent_a.append(('x', k, np.cos(th)))
                ent_b.append(('x', k, -np.sin(th)))
        else:
            # radix-4: V1 = PR - i*PI; c=0 uses V1 (f0=1), c=2 uses conj V1
            sgn = 1.0 if c == 0 else -1.0
            for c1 in range(2):
                n1 = 128 * c1 + np.arange(128, dtype=np.float64)
                th = 2 * np.pi * np.outer(n1, f) / L
                cos, sin = np.cos(th), np.sin(th)
                # planes: 0 = PR, 1 = PI (within v tile); idx = plane*2 + c1
                ent_a.append(('v', 0 * 2 + c1, cos))
                ent_a.append(('v', 1 * 2 + c1, -sgn * sin))
                ent_b.append(('v', 0 * 2 + c1, -sin))
                ent_b.append(('v', 1 * 2 + c1, -sgn * cos))
        for slot, ents in ((0, ent_a), (1, ent_b)):
            sched[(c, slot)] = [(add(mat), src, idx) for src, idx, mat in ents]
    W = np.stack(mats).astype(np.float16)
    return W, sched


_W_NP, _W_SCHED = _build_W()


def _build_G() -> np.ndarray:
    """12 inverse planes [128, 64]: idx c = k1, 4+c = k2, 8+c = k3."""
    G = np.zeros((12, 128, NL), dtype=np.float64)
    nj = (np.arange(NL) - 32).astype(np.float64)
    for c in range(4):
        for r in range(128):
            f = 4 * r + c + 1
            w = 1.0 if f == 512 else 2.0
            cosv = 16.0 * w * np.cos(2 * np.pi * f * nj / L) / L
            sinv = 16.0 * w * np.sin(2 * np.pi * f * nj / L) / L
            G[0 + c, r] = cosv - sinv     # k1 = ys1*a2
            G[4 + c, r] = sinv            # k2 = a1*ys2
            G[8 + c, r] = -cosv           # k3 = b1*yd2
    return G.astype(np.float16)


def build_bass() -> bass.Bass:
    nc = bacc.Bacc("TRN2", target_bir_lowering=False, debug=False)
    xT = nc.dram_tensor("xT", [NB, M, L, T], FP16, kind="ExternalInput")
    out = nc.dram_tensor("out", [NB, NL, NPAIRS, T], F32, kind="ExternalOutput")
    Wh = nc.inline_tensor(np.ascontiguousarray(_W_NP), name="Wmat")
    Gh = nc.inline_tensor(np.ascontiguousarray(_build_G()), name="Gmat")

    with tile.TileContext(nc) as tc, ExitStack() as ctx:
        consts = ctx.enter_context(tc.tile_pool(name="consts", bufs=1))
        xt_pool = ctx.enter_context(tc.tile_pool(name="xt", bufs=2))
        v_pool = ctx.enter_context(tc.tile_pool(name="v", bufs=2))
        # y double-buffered: batch b+1's forward/normalize overlaps batch b's
        # pairs+inverse phase
        y_pool = ctx.enter_context(tc.tile_pool(name="y", bufs=2))
        tmp_pool = ctx.enter_context(tc.tile_pool(name="tmp", bufs=2))
        r_pool = ctx.enter_context(tc.tile_pool(name="r", bufs=2))
        fwd_psum = ctx.enter_context(tc.tile_pool(name="fps", bufs=3, space="PSUM"))
        inv_psum = ctx.enter_context(tc.tile_pool(name="ips", bufs=2, space="PSUM"))

        w_sb = consts.tile([128, len(_W_NP), 128], FP16)
        nc.sync.dma_start(w_sb[:], Wh[:].rearrange("i p j -> p i j"))
        g_sb = consts.tile([128, 12, NL], FP16)
        nc.sync.dma_start(g_sb[:], Gh[:].rearrange("i p j -> p i j"))

        for b in range(NB):
            ya = [y_pool.tile([128, 4, 2, T], FP16, tag=f"ya{c}", name=f"ya{c}") for c in range(4)]
            yb = [y_pool.tile([128, 4, 2, T], FP16, tag=f"yb{c}", name=f"yb{c}") for c in range(4)]
            ys = [y_pool.tile([128, 4, 2, T], FP16, tag=f"ys{c}", name=f"ys{c}") for c in range(4)]
            yd = [y_pool.tile([128, 4, 2, T], FP16, tag=f"yd{c}", name=f"yd{c}") for c in range(4)]

            for mg in range(4):
                xt_sb = xt_pool.tile([128, 8, 2, T], FP16, tag="xt")
                # input DMAs on the ACT HWDGE queue so they are never
                # head-of-line blocked by the (late-bound) output DMAs on sync
                for mi in range(2):
                    nc.scalar.dma_start(
                        xt_sb[:, :, mi],
                        xT[b, 2 * mg + mi].rearrange("(k p) t -> p k t", p=128),
                    )
                # radix-4 half-butterflies on Pool: n2-chunk k = c1 + 2*n2
                v_sb = v_pool.tile([128, 2, 2, 2, T], FP16, tag="v")  # plane, c1, mi, t
                for c1 in range(2):
                    nc.gpsimd.tensor_sub(v_sb[:, 0, c1], xt_sb[:, c1], xt_sb[:, c1 + 4])
                    nc.gpsimd.tensor_sub(v_sb[:, 1, c1], xt_sb[:, c1 + 2], xt_sb[:, c1 + 6])
                vf = v_sb[:].rearrange("p a b c t -> p (a b) c t")

                # direct chunks first (only need the DMA), radix chunks after
                for c in (1, 3, 0, 2):
                    ps_a = fwd_psum.tile([128, 2, T], F32, tag="psa")
                    ps_b = fwd_psum.tile([128, 2, T], F32, tag="psb")
                    for ps, slot in ((ps_a, 0), (ps_b, 1)):
                        lst = _W_SCHED[(c, slot)]
                        for i, (widx, src, idx) in enumerate(lst):
                            rhs = xt_sb[:, idx] if src == 'x' else vf[:, idx]
                            nc.tensor.matmul(
                                ps[:], w_sb[:, widx], rhs,
                                start=(i == 0), stop=(i == len(lst) - 1),
                            )
                    sq_a = tmp_pool.tile([128, 2, T], F32, tag="sqa")
                    sq_b = tmp_pool.tile([128, 2, T], F32, tag="sqb")
                    w = tmp_pool.tile([128, 2, T], F32, tag="w")
                    nc.scalar.square(sq_a[:], ps_a[:])
                    nc.scalar.square(sq_b[:], ps_b[:])
                    nc.gpsimd.tensor_add(sq_a[:], sq_a[:], sq_b[:])
                    nc.scalar.activation(
                        w[:], sq_a[:],
                        mybir.ActivationFunctionType.Abs_reciprocal_sqrt,
                        scale=16.0,
                    )
                    nc.vector.tensor_mul(ya[c][:, mg], ps_a[:], w[:])
                    nc.vector.tensor_mul(yb[c][:, mg], ps_b[:], w[:])
                    nc.vector.tensor_add(ys[c][:, mg], ya[c][:, mg], yb[c][:, mg])
                    nc.vector.tensor_sub(yd[c][:, mg], ya[c][:, mg], yb[c][:, mg])

            # ---- pairs + inverse (diagonal pairing) ----
            yaf = [ya[c][:].rearrange("p a b t -> p (a b t)") for c in range(4)]
            ybf = [yb[c][:].rearrange("p a b t -> p (a b t)") for c in range(4)]
            ysf = [ys[c][:].rearrange("p a b t -> p (a b t)") for c in range(4)]
            ydf = [yd[c][:].rearrange("p a b t -> p (a b t)") for c in range(4)]
            for d in range(1, M):
                lanes = M - d
                kb = sum(M - dd for dd in range(1, d))
                for l0 in range(0, lanes, LANE_CAP):
                    lc = min(LANE_CAP, lanes - l0)
                    rows = lc * T
                    s1 = slice(l0 * T, l0 * T + rows)            # m1 side
                    s2 = slice((l0 + d) * T, (l0 + d) * T + rows)  # m2 side
                    r_sb = r_pool.tile([128, 12, LANE_CAP * T], FP16, tag="ru")
                    for c in range(4):
                        nc.vector.tensor_mul(r_sb[:, 0 + c, :rows], ysf[c][:, s1], yaf[c][:, s2])
                        nc.vector.tensor_mul(r_sb[:, 4 + c, :rows], yaf[c][:, s1], ysf[c][:, s2])
                        nc.gpsimd.tensor_mul(r_sb[:, 8 + c, :rows], ybf[c][:, s1], ydf[c][:, s2])
                    for n0 in range(0, rows, 500):
                        nn = min(500, rows - n0)
                        ps_o = inv_psum.tile([64, 500], F32, tag="ops")
                        for idx in range(12):
                            nc.tensor.matmul(
                                ps_o[:, :nn],
                                g_sb[:, idx],
                                r_sb[:, idx, ds(n0, nn)],
                                start=(idx == 0), stop=(idx == 11),
                            )
                        o_sb = tmp_pool.tile([64, 2, T], F32, tag="osb")
                        nlanes = nn // T
                        nc.scalar.copy(
                            o_sb[:, :nlanes],
                            ps_o[:, :nn].rearrange("p (l t) -> p l t", t=T),
                        )
                        nc.sync.dma_start(
                            out[b, :, ds(kb + l0 + n0 // T, nlanes)],
                            o_sb[:, :nlanes],
                        )
    nc.compile()
    return nc


_NC_CACHE = None


def kernel(x: np.ndarray) -> np.ndarray:
    global _NC_CACHE
    x = np.asarray(x, dtype=np.float32)
    assert x.shape == (B, M, T, L)
    xT = np.ascontiguousarray(x.transpose(0, 1, 3, 2)).astype(np.float16)
    s0 = np.sign(x.sum(axis=-1))  # [B, M, T] DC sign for host PHAT term
    if _NC_CACHE is None:
        _NC_CACHE = build_bass()
    nc = _NC_CACHE
    in_maps = [{"xT": xT[c * NB:(c + 1) * NB]} for c in range(NCORES)]
    trace = bool(int(os.environ.get("GCC_TRACE", "0")))
    res = run_bass_kernel_spmd(nc, in_maps, core_ids=list(range(NCORES)),
                               trace=trace)
    if trace and res.exec_time_ns is not None:
        print(f"HW exec time: {res.exec_time_ns} ns")
        if res.instructions_and_trace is not None:
            print("trace:", res.instructions_and_trace[1])
    out = np.concatenate([r["out"] for r in res.results], axis=0)  # [B,NL,28diag,T]
    plist = [m * (2 * M - m - 1) // 2 + (m + d - m - 1)
             for d in range(1, M) for m in range(M - d)]
    final = np.empty((B, NPAIRS, T, NL), dtype=np.float32)
    final[:, plist] = out.transpose(0, 2, 3, 1)
    # host DC (bin 0) PHAT term: sign(S1)*sign(S2)/L, constant over lags
    i1, i2 = np.triu_indices(M, k=1)
    final += (s0[:, i1] * s0[:, i2])[..., None].astype(np.float32) / L
    return final
